# revision 49
# baseline (speedup 1.0000x reference)
"""GCNConv (D^-1/2 A D^-1/2 X W + b) on 8 Trainium2 NeuronCores.

Strategy (row-sharded over nodes, per the sharding hint):
  - each core owns a [1024, 8192] row block of the adjacency; weight/bias
    replicated.
  - the adjacency block is streamed ONCE from HBM (f32, 1MB chunks, HWDGE);
    per chunk it is cast to bf16 on ACT with the row-sum fused via accum_out,
    transposed on TensorE, and copied PSUM->SBUF on DVE into a resident bf16
    A^T (16.8MB SBUF).
  - support rows S = X @ W (bf16) either per-core + AllGathered (kagmode
    "sv"/"early") or computed fully per core from replicated X ("repx",
    leaving only 8 tiny 512-byte d AllGathers as collectives).
  - after each 128-row m-tile i: d_i = rsqrt(deg+l) and the SV slab for
    phase i becomes available (AG or local scale).
  - the MAIN matmul is interleaved with the stream: matmul (k-tile t,
    own m-tile i') is emitted once A^T of m-tile i' exists and the SV slab
    for phase(t) is available, in small quanta between transpose quads, so
    TensorE overlaps the stream instead of trailing it.  PSUM: 4 banks
    accumulate out[m-tile], 4 banks rotate for transposes.
  - epilogue scales rows by d_m, adds broadcast bias, stores f32; emitted
    inline right after each m-tile's last matmul.

Tuning state (2026-08-10): KQUANT=8 + KCHUNK=512/KNATF=8/KNATB=8 measured
fastest in same-process A/B (fine chunks with 8-deep double buffering on
both stream stages: -5.2us/2.6% vs chunk-2048; quant 8 vs 6: -3us).  All
other scheduling variants (queue moves, AG deferral, inline part-A, tmode,
wide-oct copies, split casts, sv2 collectives) were ties or losses -- the
pipeline is latency-coupled with balanced engine load (DMA ~111us, DVE
~96, PE ~94, ACT ~71 per TimelineSim).
"""
import sys
sys.path.insert(0, "/opt/trn_rl_repo")
import os as _os
from contextlib import ExitStack

import numpy as np

import concourse.bass as bass
import concourse.bacc as bacc
import concourse.tile as tile
import concourse.bass_utils as bass_utils
import concourse.mybir as mybir

N_CORES = 8
N = 8192
DIN = 256
DOUT = 256
P = 128
M_LOC = N // N_CORES          # 1024 rows per core
MT = M_LOC // P               # 8 m-tiles per core
KT = N // P                   # 64 k-tiles global
CHUNK = 2048                  # k-chunk per streaming DMA
NCH = N // CHUNK              # 4 chunks per m-tile
NQ = CHUNK // (4 * P)         # 4 k-quads per chunk
GT = KT // 4                  # 16 k-quad groups (atp tiles per m-tile)
F32 = mybir.dt.float32
BF16 = mybir.dt.bfloat16
RG = [list(range(N_CORES))]
Alu = mybir.AluOpType
ActF = mybir.ActivationFunctionType
AxX = mybir.AxisListType.X

CFG = dict(
    quant=int(_os.environ.get("KQUANT", "8")),   # matmuls per transpose quad
    delay=int(_os.environ.get("KDELAY", "0")),   # defer AG-dep mms one m-tile
    kcast=_os.environ.get("KCAST", "act"),       # act | split
    kcopy=_os.environ.get("KCOPY", "dve"),       # dve | split3 | split2
    ktmode=_os.environ.get("KTMODE", "mm"),      # mm | tmode
    kagmode=_os.environ.get("KAGMODE", "sv"),    # sv | early | repx
    kstream=_os.environ.get("KSTREAM", "act"),   # act | cast (fused cast-DMA)
    ksvq=_os.environ.get("KSVQ", "sync"),        # sync | act | mix (svag queues)
    kagdefer=int(_os.environ.get("KAGDEFER", "0")),  # defer AG glue into next tile
    koutq=_os.environ.get("KOUTQ", "sync"),      # sync | act (epilogue store q)
    kinlinea=int(_os.environ.get("KINLINEA", "0")),  # part-A mms pushed per quad
    kaslack=int(_os.environ.get("KASLACK", "2")),    # quads of dep slack for A
    knatb=int(_os.environ.get("KNATB", "8")),        # override natbp bufs
    knatf=int(_os.environ.get("KNATF", "8")),        # natp (f32 chunk) bufs
    kchunk=int(_os.environ.get("KCHUNK", "512")),   # stream chunk width
    krsqrt=int(_os.environ.get("KRSQRT", "0")),      # 1: 2-op recip+sqrt d
    kwide=int(_os.environ.get("KWIDE", "0")),        # tmode: 8-transpose octs
)


def _emit_body(nc, tc, pools, consts, rep, stage, cfg):
    do_transp = stage in ("transp", "nomm", "full")
    do_coll = stage in ("nomm", "full")
    do_mm = stage == "full"
    agm = cfg["kagmode"]
    tmode = cfg["ktmode"] == "tmode"
    (natp, natbp, supp, xtp, atpp, svp, dtp, stagep, tpp, tpb, mmp,
     dram) = pools
    (ident, wb, bias_bc, lv, a, x, w, bias, out) = consts
    R = f"r{rep}_"

    # ---- DRAM bounce buffers for the collectives ----
    if agm in ("early", "repx"):
        # d rides as ONE partition row so every DMA is a single contiguous
        # 512B descriptor (a [128,1] partition-major write would be 128
        # four-byte descriptors -- pathological)
        dag_in = [dram.tile([1, P], F32, tag=f"dagin{i}", name=R + f"dagin{i}")
                  for i in range(MT)]
        dag_out = [dram.tile([N_CORES, P], F32, addr_space="Shared",
                             tag=f"dagout{i}", name=R + f"dagout{i}")
                   for i in range(MT)]
    if agm == "early":
        sag_in = dram.tile([M_LOC, DOUT], BF16, tag="sagin", name=R + "sagin")
        sag_out = dram.tile([N, DOUT], BF16, addr_space="Shared",
                            tag="sagout", name=R + "sagout")
    if agm == "sv":
        svag_in = [dram.tile([P, DOUT], BF16, tag=f"svin{i}",
                             name=R + f"svin{i}") for i in range(MT)]
        svag_out = [dram.tile([N_CORES * P, DOUT], BF16, addr_space="Shared",
                              tag=f"svout{i}", name=R + f"svout{i}")
                    for i in range(MT)]
    if agm == "sv2":
        svag_in = [dram.tile([2 * P, DOUT], BF16, tag=f"svin{i}",
                             name=R + f"svin{i}") for i in range(MT // 2)]
        svag_out = [dram.tile([N_CORES * 2 * P, DOUT], BF16,
                              addr_space="Shared", tag=f"svout{i}",
                              name=R + f"svout{i}") for i in range(MT // 2)]

    # ---- resident transposed adjacency (bf16) and SV slabs ----
    wide = bool(cfg["kwide"]) and tmode
    atp = {}
    if wide:
        for o in range(GT // 2):
            for i in range(MT):
                atp[(o, i)] = atpp.tile([P, 1024], BF16, tag="atp",
                                        name=R + f"atpw_{o}_{i}")
    else:
        for g in range(GT):
            for i in range(MT):
                atp[(g, i)] = atpp.tile([P, 512], BF16, tag="atp",
                                        name=R + f"atp_{g}_{i}")
    sv = [svp.tile([P, N_CORES * DOUT], BF16, tag="sv", name=R + f"sv{i}")
          for i in range(MT)]

    # ---- prologue: own support rows S = X @ W (sv/early modes) ----
    if agm in ("sv", "sv2", "early") and stage != "dmaonly":
        xt = [xtp.tile([P, M_LOC], BF16, tag="xt", name=R + f"xt{dt}")
              for dt in range(DIN // P)]
        for i in range(MT):
            xb = supp.tile([P, DIN], BF16, tag="xb", name=R + f"xb{i}")
            nc.gpsimd.dma_start(xb[:], x.ap()[i * P:(i + 1) * P, :])
            for dt in range(DIN // P):
                ps = tpp.tile([P, 512], F32, tag="tp", name=R + f"xps{i}_{dt}")
                nc.tensor.matmul(ps[:, 0:P], xb[:, dt * P:(dt + 1) * P],
                                 ident[:], start=True, stop=True)
                nc.vector.tensor_copy(xt[dt][:, i * P:(i + 1) * P],
                                      ps[:, 0:P])
        sown = []
        for i in range(MT):
            sps_t = tpp.tile([P, 512], F32, tag="tp", name=R + f"sps{i}")
            sps = sps_t[:, 0:DOUT]
            for dt in range(DIN // P):
                nc.tensor.matmul(sps, xt[dt][:, i * P:(i + 1) * P], wb[dt][:],
                                 start=(dt == 0), stop=(dt == DIN // P - 1))
            sst = supp.tile([P, DOUT], BF16, tag="sown", name=R + f"sown{i}")
            nc.scalar.copy(sst[:], sps)
            sown.append(sst)
            if agm == "early" and do_coll:
                nc.sync.dma_start(sag_in[i * P:(i + 1) * P, :], sst[:])
        if agm == "early" and do_coll:
            # one S AllGather up front; slabs land unscaled in sv[i]
            nc.gpsimd.collective_compute(
                "AllGather", Alu.bypass, replica_groups=RG,
                ins=[sag_in.opt()], outs=[sag_out.opt()])
            sag_view = sag_out[:].rearrange("(r il p) n -> il p r n",
                                            r=N_CORES, il=MT, p=P)
            for i in range(MT):
                nc.gpsimd.dma_start(sv[i][:], sag_view[i])

    # ---- repx: full S computed locally; emitted interleaved via sblocks ----
    XPH = 4                   # r's per X piece
    NXP = N_CORES // XPH      # pieces per slab
    x_view = None
    if agm == "repx":
        x_view = x.ap().rearrange("(rh rr il p) n -> il p rh rr n",
                                  rh=NXP, rr=XPH, il=MT, p=P)

    def emit_sblock(pc):
        # load X rows for piece pc (slab s = pc//NXP, r's rh*XPH..), compute
        # S[t] = X[t] @ W for each, write into sv[s] slab (unscaled bf16).
        # gpsimd-issued + double-buffered so the A-stream queue never blocks.
        s, rh = pc // NXP, pc % NXP
        xp = xtp.tile([P, XPH * DIN], F32, tag="xp", name=R + f"xp{pc}")
        nc.gpsimd.dma_start(xp[:], x_view[s][:, rh])
        for rr in range(XPH):
            r = rh * XPH + rr
            tps = tpp.tile([P, 512], F32, tag="tp", name=R + f"xtp{pc}_{rr}")
            for dt in range(DIN // P):
                src = xp[:, rr * DIN + dt * P:rr * DIN + (dt + 1) * P]
                nc.tensor.matmul(tps[:, dt * P:(dt + 1) * P], src, identf[:],
                                 start=True, stop=True, is_transpose=True)
            xts = supp.tile([P, DIN], BF16, tag="xts", bufs=3,
                            name=R + f"xts{pc}_{rr}")
            nc.vector.tensor_copy(xts[:], tps[:, 0:DIN])
            sps_t = tpp.tile([P, 512], F32, tag="tp", name=R + f"srp{pc}_{rr}")
            sps = sps_t[:, 0:DOUT]
            for dt in range(DIN // P):
                nc.tensor.matmul(sps, xts[:, dt * P:(dt + 1) * P], wb[dt][:],
                                 start=(dt == 0), stop=(dt == DIN // P - 1))
            nc.vector.tensor_copy(sv[s][:, r * DOUT:(r + 1) * DOUT], sps)

    identf = None
    if agm == "repx":
        identf = supp.tile([P, P], F32, tag="identf", bufs=1,
                           name=R + "identf")
        nc.vector.tensor_copy(identf[:], ident[:])

    CH = cfg["kchunk"]
    NCHl = N // CH                # chunks per m-tile
    NQl = CH // (4 * P)           # quads per chunk
    par = dtp.tile([P, MT * NCHl], F32, tag="par", name=R + "par")
    dcols = dtp.tile([P, MT], F32, tag="dcols", name=R + "dcols")

    # ---- main-matmul scheduler state ----
    mmps = [mmp.tile([P, 512], F32, tag="mmps", name=R + f"mmps_{b}")
            for b in range(MT // 2)]
    pending = []
    deferred = []
    bank_started = [False] * (MT // 2)
    mm_left = [KT] * MT

    def emit_epilogue(ip):
        src = mmps[ip // 2][:, (ip % 2) * DOUT:(ip % 2 + 1) * DOUT]
        st1 = stagep.tile([P, DOUT], F32, tag="stage", name=R + f"st1_{ip}")
        nc.vector.tensor_scalar_mul(st1[:], src, dcols[:, ip:ip + 1])
        st2 = stagep.tile([P, DOUT], F32, tag="stage", name=R + f"st2_{ip}")
        nc.vector.tensor_add(st2[:], st1[:], bias_bc[:])
        outq = nc.scalar if cfg["koutq"] == "act" else nc.sync
        outq.dma_start(out.ap()[ip * P:(ip + 1) * P, :], st2[:])

    def emit_mm(t, ip):
        ph, r = t % MT, t // MT
        b = ip // 2
        first = not bank_started[b]
        bank_started[b] = True
        dst = mmps[b][:, (ip % 2) * DOUT:(ip % 2 + 1) * DOUT]
        # start=True clears the WHOLE bank's has_written bits, so only the
        # bank's first matmul (even half) may carry it; the odd half's first
        # matmul overwrites via the cleared bits.
        if wide:
            lhs = atp[(t // 8, ip)][:, (t % 8) * P:(t % 8 + 1) * P]
        else:
            lhs = atp[(t // 4, ip)][:, (t % 4) * P:(t % 4 + 1) * P]
        nc.tensor.matmul(
            dst,
            lhs,
            sv[ph][:, r * DOUT:(r + 1) * DOUT],
            start=first, stop=(mm_left[ip] == 1),
            skip_group_check=True)
        mm_left[ip] -= 1
        if mm_left[ip] == 0:
            emit_epilogue(ip)

    def emit_some(budget):
        while budget > 0 and pending:
            emit_mm(*pending.pop(0))
            budget -= 1

    def emit_copy(g, dst, ps):
        if cfg["kcopy"] == "dve":
            nc.vector.tensor_copy(dst, ps)
        elif cfg["kcopy"] == "split2":
            if g % 2 == 1:
                nc.scalar.copy(dst, ps)
            else:
                nc.vector.tensor_copy(dst, ps)
        else:  # split3
            if g % 3 == 2:
                nc.scalar.copy(dst, ps)
            else:
                nc.vector.tensor_copy(dst, ps)

    # ---- stream the adjacency block ----
    svq = nc.scalar if cfg["ksvq"] in ("act", "mix") else nc.sync
    rbq = nc.scalar if cfg["ksvq"] == "act" else nc.gpsimd
    pending_coll = []

    def emit_sv_coll(i):
        # svag write waits on the deg->d->scale chain; AG readback waits on
        # the collective.  Emitting these on the stream queues stalls the
        # stream, so they ride the (nearly idle) scalar queue and can be
        # deferred into the next m-tile's chunk loop.
        svo = supp.tile([P, DOUT], BF16, tag="svo", bufs=2,
                        name=R + f"svo{i}")
        nc.vector.tensor_scalar_mul(svo[:], sown[i][:], dcols[:, i:i + 1])
        svq.dma_start(svag_in[i][:], svo[:])
        if cfg.get("simcc"):
            # collective stand-in for single-core TimelineSim: readback pulls
            # the same byte volume from DRAM, no cross-core semantics
            for r in range(N_CORES):
                rbq.dma_start(sv[i][:, r * DOUT:(r + 1) * DOUT],
                              svag_in[i][:])
        else:
            nc.gpsimd.collective_compute(
                "AllGather", Alu.bypass, replica_groups=RG,
                ins=[svag_in[i].opt()], outs=[svag_out[i].opt()])
            rbq.dma_start(
                sv[i][:],
                svag_out[i][:].rearrange("(r p) n -> p r n",
                                         r=N_CORES, p=P))

    prev_ph = 0
    for i in range(MT):
        deg = dtp.tile([P, 1], F32, tag="deg", bufs=2, name=R + f"deg{i}")
        for j in range(NCHl):
            c = i * NCHl + j
            if stage == "dmaonly":
                natf = natp.tile([P, CH], F32, tag="nat",
                                 name=R + f"natf{i}_{j}")
                nc.sync.dma_start(
                    natf[:],
                    a.ap()[i * P:(i + 1) * P, j * CH:(j + 1) * CH])
                continue
            if cfg["kstream"] == "cast":
                # fused cast-DMA: HBM f32 -> SBUF bf16 on SWDGE; rowsum
                # moves to a DVE reduce on the bf16 tile
                nat = natbp.tile([P, CH], BF16, tag="natb",
                                 name=R + f"nat{i}_{j}")
                nc.gpsimd.dma_start(
                    nat[:],
                    a.ap()[i * P:(i + 1) * P, j * CH:(j + 1) * CH])
                nc.vector.tensor_reduce(par[:, c:c + 1], nat[:], axis=AxX,
                                        op=Alu.add)
            else:
                natf = natp.tile([P, CH], F32, tag="nat",
                                 name=R + f"natf{i}_{j}")
                nc.sync.dma_start(
                    natf[:],
                    a.ap()[i * P:(i + 1) * P, j * CH:(j + 1) * CH])
                nat = natbp.tile([P, CH], BF16, tag="natb",
                                 name=R + f"nat{i}_{j}")
                if cfg["kcast"] == "act" or j % 2 == 0:
                    nc.scalar.activation(nat[:], natf[:], ActF.Copy,
                                         accum_out=par[:, c:c + 1])
                else:
                    nc.vector.tensor_scalar(nat[:], natf[:], 1.0, None,
                                            op0=Alu.mult, op1=Alu.add,
                                            accum_out=par[:, c:c + 1])
            if pending_coll and j == 1:
                for ic in pending_coll:
                    emit_sv_coll(ic)
                pending_coll = []
            for q in (range(NQl) if do_transp else ()):
                g = j * NQl + q
                if wide:
                    # 8 transposes share one full-bank bf16 PSUM tile; one
                    # [P,1024] 2x-mode DVE copy per oct instead of two
                    if g % 2 == 0:
                        ps_oct = tpb.tile([P, 1024], BF16, tag="tpb",
                                          name=R + f"tpo{i}_{g // 2}")
                    base = (g % 2) * 512
                    for u in range(4):
                        s = q * 4 + u
                        nc.tensor.matmul(ps_oct[:, base + u * P:
                                                base + (u + 1) * P],
                                         nat[:, s * P:(s + 1) * P], ident[:],
                                         start=True, stop=True,
                                         is_transpose=True)
                    if g % 2 == 1:
                        emit_copy(g, atp[(g // 2, i)][:], ps_oct[:])
                else:
                    if tmode:
                        ps = tpb.tile([P, 512], BF16, tag="tpb",
                                      name=R + f"tps{i}_{g}")
                    else:
                        ps = tpp.tile([P, 512], F32, tag="tp",
                                      name=R + f"tps{i}_{g}")
                    for u in range(4):
                        s = q * 4 + u
                        nc.tensor.matmul(ps[:, u * P:(u + 1) * P],
                                         nat[:, s * P:(s + 1) * P], ident[:],
                                         start=True, stop=True,
                                         is_transpose=tmode)
                    emit_copy(g, atp[(g, i)][:], ps[:])
                if do_mm:
                    # part A inline: mms of already-available phases that
                    # only needed a freshly-transposed atp tile.  Pushed with
                    # kaslack quads of delay so the DVE copy they depend on
                    # has drained before TensorE pops them (avoids clogging
                    # the 4-deep PE wait queue).
                    if cfg["kinlinea"]:
                        gd = g - cfg["kaslack"]
                        if gd >= 0:
                            for u in range(4):
                                t = gd * 4 + u
                                if t % MT < prev_ph:
                                    pending.append((t, i))
                    emit_some(cfg["quant"])
            if agm == "repx" and c < MT * NXP:
                emit_sblock(c)
        if stage == "dmaonly":
            continue
        # ---- end of m-tile i: degree -> d_i ----
        nc.vector.tensor_reduce(deg[:], par[:, i * NCHl:(i + 1) * NCHl],
                                axis=AxX, op=Alu.add)
        deg2 = dtp.tile([P, 1], F32, tag="deg2", bufs=2, name=R + f"deg2{i}")
        nc.vector.tensor_scalar_add(deg2[:], deg[:], lv[:])
        if cfg["krsqrt"]:
            # d = sqrt(1/deg2): 2-op chain (DVE reciprocal is accurate; ACT
            # Sqrt table error ~1e-3 is far inside the 2e-2 gate)
            rr = dtp.tile([P, 1], F32, tag="rr", bufs=2, name=R + f"rr{i}")
            nc.vector.reciprocal(rr[:], deg2[:])
            nc.scalar.sqrt(dcols[:, i:i + 1], rr[:])
        else:
            s0 = dtp.tile([P, 1], F32, tag="s0", bufs=2, name=R + f"s0{i}")
            nc.scalar.sqrt(s0[:], deg2[:])
            r0 = dtp.tile([P, 1], F32, tag="r0", bufs=2, name=R + f"r0{i}")
            nc.vector.reciprocal(r0[:], s0[:])
            # one Newton step: d = r0 * (1.5 - 0.5 * deg2 * r0^2)
            t1 = dtp.tile([P, 1], F32, tag="t1", bufs=2, name=R + f"t1{i}")
            nc.vector.tensor_mul(t1[:], r0[:], r0[:])
            t2 = dtp.tile([P, 1], F32, tag="t2", bufs=2, name=R + f"t2{i}")
            nc.vector.tensor_mul(t2[:], t1[:], deg2[:])
            t3 = dtp.tile([P, 1], F32, tag="t3", bufs=2, name=R + f"t3{i}")
            nc.vector.tensor_scalar(t3[:], t2[:], -0.5, 1.5, op0=Alu.mult,
                                    op1=Alu.add)
            nc.vector.tensor_mul(dcols[:, i:i + 1], r0[:], t3[:])
        if do_coll and agm in ("early", "repx"):
            # tiny d AllGather; scale the resident S slab in place
            nc.gpsimd.dma_start(dag_in[i][:], dcols[:, i:i + 1])
            nc.gpsimd.collective_compute(
                "AllGather", Alu.bypass, replica_groups=RG,
                ins=[dag_in[i].opt()], outs=[dag_out[i].opt()])
            dsl = dtp.tile([P, N_CORES], F32, tag=f"dsl{i}",
                           name=R + f"dsl{i}")
            nc.gpsimd.dma_start(
                dsl[:], dag_out[i][:].rearrange("(r p) o -> p r o",
                                                r=N_CORES, p=P))
            for r in range(N_CORES):
                seg = sv[i][:, r * DOUT:(r + 1) * DOUT]
                nc.vector.tensor_scalar_mul(seg, seg, dsl[:, r:r + 1])
        elif do_coll and agm == "sv":
            # own SV rows, scaled locally, AllGathered per m-tile
            if cfg["kagdefer"] and i + 1 < MT:
                pending_coll.append(i)
            else:
                emit_sv_coll(i)
        elif do_coll and agm == "sv2":
            # AG every second m-tile, two scaled SV tiles per payload
            svo = supp.tile([P, DOUT], BF16, tag=f"svo{i % 2}", bufs=1,
                            name=R + f"svo{i}")
            nc.vector.tensor_scalar_mul(svo[:], sown[i][:], dcols[:, i:i + 1])
            cc = i // 2
            nc.sync.dma_start(svag_in[cc][(i % 2) * P:(i % 2 + 1) * P, :],
                              svo[:])
            if i % 2 == 1:
                nc.gpsimd.collective_compute(
                    "AllGather", Alu.bypass, replica_groups=RG,
                    ins=[svag_in[cc].opt()], outs=[svag_out[cc].opt()])
                v = svag_out[cc][:].rearrange("(r ii p) n -> ii p r n",
                                              r=N_CORES, ii=2, p=P)
                nc.gpsimd.dma_start(sv[i - 1][:], v[0])
                nc.gpsimd.dma_start(sv[i][:], v[1])
        if do_mm:
            # newly-ready matmuls: old phases x new atp first (they need no
            # fresh collective), then new phases across all ready atp
            if agm == "sv2":
                ph_avail = i + 1 if i % 2 == 1 else i
            elif agm == "sv" and cfg["kagdefer"]:
                # glue for phase p is emitted at m-tile p+1 chunk 1, so the
                # phase only becomes usable after m-tile p+1 completes
                ph_avail = i if i + 1 < MT else MT
            else:
                ph_avail = i + 1
            atp_avail = i + 1
            if cfg["delay"]:
                pending.extend(deferred)
                deferred = []
            # part A stragglers still inside the slack window
            if cfg["kinlinea"]:
                for g2 in range(max(0, GT - cfg["kaslack"]), GT):
                    for u in range(4):
                        t = g2 * 4 + u
                        if t % MT < prev_ph:
                            pending.append((t, i))
            else:
                # original behavior: all of part A released at tile end
                for ph in range(min(prev_ph, ph_avail)):
                    for r in range(N_CORES):
                        pending.append((r * MT + ph, i))
            # (other part A pushed inline per transposed quad above)
            # part B: newly-available phases x all available atp
            newph = [(r * MT + ph, ip)
                     for ph in range(prev_ph, ph_avail)
                     for r in range(N_CORES)
                     for ip in range(atp_avail)]
            prev_ph = ph_avail
            if cfg["delay"]:
                deferred.extend(newph)
            else:
                pending.extend(newph)

    # ---- drain remaining matmuls (epilogues fire inline per m-tile) ----
    if do_mm:
        pending.extend(deferred)
        emit_some(len(pending))
        assert all(v == 0 for v in mm_left)
    else:
        # partial-stage builds still need the output written
        st = stagep.tile([P, DOUT], F32, tag="stage", name=R + "stz")
        nc.vector.memset(st[:], 0.0)
        for i in range(MT):
            nc.sync.dma_start(out.ap()[i * P:(i + 1) * P, :], st[:])


def build(repeat=1, stage="full", **over):
    cfg = dict(CFG)
    cfg.update(over)
    agm = cfg["kagmode"]
    nc = bacc.Bacc("TRN2", target_bir_lowering=False, debug=False,
                   num_devices=N_CORES)
    a = nc.dram_tensor("a", [M_LOC, N], F32, kind="ExternalInput")
    x = nc.dram_tensor("x", [N if agm == "repx" else M_LOC, DIN], F32,
                       kind="ExternalInput")
    w = nc.dram_tensor("w", [DIN, DOUT], F32, kind="ExternalInput")
    bias = nc.dram_tensor("bias", [DOUT], F32, kind="ExternalInput")
    lvec = nc.dram_tensor("lvec", [P, 1], F32, kind="ExternalInput")
    out = nc.dram_tensor("out", [M_LOC, DOUT], F32, kind="ExternalOutput")

    with tile.TileContext(nc) as tc, ExitStack() as ctx:
        cpool = ctx.enter_context(tc.tile_pool(name="cpool", bufs=1))
        natp = ctx.enter_context(tc.tile_pool(name="natp",
                                              bufs=cfg["knatf"]))
        natbp = ctx.enter_context(tc.tile_pool(
            name="natbp", bufs=cfg["knatb"] or
            (4 if cfg["kstream"] == "cast" else 3)))
        supp = ctx.enter_context(tc.tile_pool(name="supp", bufs=MT))
        xtp = ctx.enter_context(tc.tile_pool(name="xtp", bufs=2))
        wide = bool(cfg["kwide"]) and cfg["ktmode"] == "tmode"
        atpp = ctx.enter_context(tc.tile_pool(
            name="atpp", bufs=(GT // 2) * MT if wide else GT * MT))
        svp = ctx.enter_context(tc.tile_pool(name="svp", bufs=MT))
        dtp = ctx.enter_context(tc.tile_pool(name="dtp", bufs=1))
        stagep = ctx.enter_context(tc.tile_pool(name="stagep", bufs=2))
        tmode = cfg["ktmode"] == "tmode"
        tpp = ctx.enter_context(tc.tile_pool(name="tpp",
                                             bufs=2 if tmode else 4,
                                             space="PSUM"))
        tpb = (ctx.enter_context(tc.tile_pool(name="tpb", bufs=2,
                                              space="PSUM"))
               if tmode else None)
        mmp = ctx.enter_context(tc.tile_pool(name="mmp", bufs=MT // 2,
                                             space="PSUM"))
        dram = ctx.enter_context(tc.tile_pool(name="dram", bufs=1,
                                              space="DRAM"))

        # ---- constants ----
        ones_bf = cpool.tile([P, P], BF16)
        nc.vector.memset(ones_bf[:], 1.0)
        ident = cpool.tile([P, P], BF16)
        nc.gpsimd.affine_select(
            ident[:], ones_bf[:], pattern=[[1, P]],
            compare_op=Alu.is_equal, fill=0.0, base=0, channel_multiplier=-1)
        wb = []
        for dt in range(DIN // P):
            wt = cpool.tile([P, DOUT], BF16, tag=f"wb{dt}", name=f"wb{dt}")
            nc.gpsimd.dma_start(wt[:], w.ap()[dt * P:(dt + 1) * P, :])
            wb.append(wt)
        lv = cpool.tile([P, 1], F32, tag="lv")
        nc.scalar.dma_start(lv[:], lvec.ap())
        # broadcast bias over partitions with a K=1 matmul
        ones_row = cpool.tile([1, P], F32, tag="ones_row")
        nc.vector.memset(ones_row[:], 1.0)
        bias_row = cpool.tile([1, DOUT], F32, tag="bias_row")
        nc.scalar.dma_start(bias_row[:], bias.ap()[None, :])
        bias_bc = cpool.tile([P, DOUT], F32, tag="bias_bc")
        bps = tpp.tile([P, 512], F32, tag="tp", name="bias_ps")
        nc.tensor.matmul(bps[:, 0:DOUT], ones_row[:], bias_row[:],
                         start=True, stop=True)
        nc.vector.tensor_copy(bias_bc[:], bps[:, 0:DOUT])

        pools = (natp, natbp, supp, xtp, atpp, svp, dtp, stagep, tpp, tpb,
                 mmp, dram)
        consts = (ident, wb, bias_bc, lv, a, x, w, bias, out)
        for rep in range(repeat):
            _emit_body(nc, tc, pools, consts, rep, stage, cfg)
    nc.compile()
    return nc


def make_in_maps(adjacency, input_feature, weight, bias, l, kagmode=None):
    if kagmode is None:
        kagmode = CFG["kagmode"]
    adjacency = np.ascontiguousarray(np.asarray(adjacency, dtype=np.float32))
    input_feature = np.ascontiguousarray(
        np.asarray(input_feature, dtype=np.float32))
    weight = np.ascontiguousarray(np.asarray(weight, dtype=np.float32))
    bias_np = np.ascontiguousarray(np.asarray(bias, dtype=np.float32))
    lval = float(np.asarray(l))
    lv = np.full((P, 1), lval, dtype=np.float32)
    in_maps = []
    for c in range(N_CORES):
        in_maps.append({
            "a": adjacency[c * M_LOC:(c + 1) * M_LOC, :],
            "x": (input_feature if kagmode == "repx"
                  else input_feature[c * M_LOC:(c + 1) * M_LOC, :]),
            "w": weight,
            "bias": bias_np,
            "lvec": lv,
        })
    return in_maps


_NC_CACHE = None


def kernel(adjacency, input_feature, weight, bias, l):
    global _NC_CACHE
    if _NC_CACHE is None:
        _NC_CACHE = build()
    nc = _NC_CACHE
    in_maps = make_in_maps(adjacency, input_feature, weight, bias, l)
    res = None
    last_err = None
    for attempt in range(3):
        try:
            res = bass_utils.run_bass_kernel_spmd(
                nc, in_maps, core_ids=list(range(N_CORES)))
            break
        except Exception as e:           # transient device wedge: retry
            last_err = e
            import time as _time
            _time.sleep(5.0 * (attempt + 1))
    if res is None:
        raise last_err
    blocks = [res.results[c]["out"] for c in range(N_CORES)]
    return np.ascontiguousarray(np.concatenate(blocks, axis=0),
                                dtype=np.float32)


if __name__ == "__main__":
    rng = np.random.default_rng(0)
    A = rng.random((N, N), dtype=np.float32)
    X = rng.standard_normal((N, DIN)).astype(np.float32)
    W = (rng.standard_normal((DIN, DOUT)) / np.sqrt(DIN)).astype(np.float32)
    B = np.zeros((DOUT,), dtype=np.float32)
    out = kernel(A, X, W, B, 1)
    deg = A.sum(axis=1) + 1.0
    d = np.where(deg > 0, deg ** -0.5, 0.0).astype(np.float32)
    ref = (A * d[:, None] * d[None, :]) @ (X @ W) + B
    err = np.abs(out - ref)
    rel = np.linalg.norm(out - ref) / np.linalg.norm(ref)
    print(f"max abs err {err.max():.3e}  rel l2 {rel:.3e}")

